# revision 21
# baseline (speedup 1.0000x reference)
"""Trainium2 Bass kernel for nn_Decoder (LSTM decoder over encoder features).

Math (per reference):
    feats = enc @ W_proj + b_proj            [B, T, DF]
    word  = embed[start_ids]                 [B, DW]   (constant per step)
    x_t   = concat(feats_t, word)
    gates = x_t @ W_ih.T + h @ W_hh.T + b    -> LSTM cell -> h_t (output)

Kernel strategy (8 cores, data-parallel over batch, B_local = 64):
  Everything on-device is kept "gate-major" (transposed: [dims, batch]) so no
  on-device transposes are ever needed:
    GEMM1: feats.T[DF, (t,b)] = W_proj(lhsT) @ enc.T(rhs)       streamed N=512
    GEMM2: XG.T[4H, (t,b)]    = W_x.T(lhsT) @ feats.T(rhs)
            + bias via a selector matmul (bias_pad(lhsT) @ S(rhs))
    rec_t: gates.T[4H, b]    += W_hh.T(lhsT) @ h.T(rhs)         into PSUM
    eltwise: sigmoid/tanh on ACT, mul/add on DVE, c kept fp32.
  The word-embedding gather, the W_proj/W_ih split, the gate-row permutation
  and all layout transposes are host-side precomputation (numpy only).

  Gate rows are permuted to [i0 f0 o0 g0 i1 f1 o1 g1] (256-row blocks) so each
  half-step's sigmoid operand (i,f,o) is one contiguous ACT instruction.

Output is written transposed (hT[p, j, t, b]) and untransposed on the host.
"""

import numpy as np
import ml_dtypes

BF16 = ml_dtypes.bfloat16

# Problem dims (hardcoded per spec)
NCORES = 8
B, T, DE, DF, DW, H, V = 512, 80, 1024, 512, 512, 512, 10000
G4 = 4 * H                      # 2048 gate rows
BL = B // NCORES                # 64 batch per core
CH = 8                          # timesteps per GEMM chunk
NCH = T // CH                   # 10 chunks
KDE = DE // 128                 # 8  contraction chunks for GEMM1
KDF = DF // 128                 # 4  contraction chunks for GEMM2
KH = H // 128                   # 4  contraction chunks for recurrence
MT = G4 // 128                  # 16 gate-row tiles
NCOL = CH * BL                  # 512 (t,b) columns per chunk

_COMPILED = None


def _build():
    import concourse.bacc as bacc
    import concourse.tile as tile
    import concourse.mybir as mybir
    import concourse.bass as bass

    dt = mybir.dt
    f32, b16 = dt.float32, dt.bfloat16
    AF = mybir.ActivationFunctionType
    ALU = mybir.AluOpType

    nc = bacc.Bacc("TRN2", target_bir_lowering=False, debug=False,
                   num_devices=NCORES)

    ident_d = nc.dram_tensor("ident", [128, 128], b16, kind="ExternalInput")
    encT_d = nc.dram_tensor("encT", [128, KDE, T * BL], b16, kind="ExternalInput")
    wproj_d = nc.dram_tensor("wproj", [128, KDE, DF], b16, kind="ExternalInput")
    wx_d = nc.dram_tensor("wx", [128, KDF, G4], b16, kind="ExternalInput")
    whh_d = nc.dram_tensor("whh", [128, KH, G4], b16, kind="ExternalInput")
    biasr_d = nc.dram_tensor("biasr", [128, MT * NCOL], b16, kind="ExternalInput")
    hT_d = nc.dram_tensor("hT", [128, KH, T, BL], f32, kind="ExternalOutput")

    with tile.TileContext(nc) as tc:
        with (
            tc.tile_pool(name="wpool", bufs=1) as wp,
            tc.tile_pool(name="encp", bufs=3) as encp,
            tc.tile_pool(name="featsp", bufs=3) as fp,
            tc.tile_pool(name="xgp", bufs=2) as xgp,
            tc.tile_pool(name="statep", bufs=1) as stp,
            tc.tile_pool(name="ewp", bufs=10) as ewp,
            tc.tile_pool(name="houtp", bufs=2) as hop,
            tc.tile_pool(name="psg", bufs=2, space=bass.MemorySpace.PSUM) as psg,
            tc.tile_pool(name="ps1", bufs=1, space=bass.MemorySpace.PSUM) as ps1,
            tc.tile_pool(name="ps2", bufs=3, space=bass.MemorySpace.PSUM) as ps2,
        ):
            # ---- persistent weights ----
            wproj_sb = wp.tile([128, KDE * DF], b16)
            wx_sb = wp.tile([128, KDF * G4], b16)
            whh_sb = wp.tile([128, KH * G4], b16)
            biasr_sb = wp.tile([128, MT * NCOL], b16)
            ident_sb = wp.tile([128, 128], b16)
            nc.sync.dma_start(ident_sb[:], ident_d[:])
            nc.sync.dma_start(wproj_sb[:], wproj_d[:])

            # warmup: keep PE busy (and HAM un-throttled) while DMAs land
            warm_ps = ps2.tile([128, 128], f32, tag="g2", name="warm")
            for _ in range(40):
                nc.tensor.matmul(warm_ps[:], ident_sb[:], ident_sb[:],
                                 start=True, stop=True)

            # ---- state (split per half so Tile deps don't serialize halves) ----
            c_half = [stp.tile([128, 128], f32, name=f"c{i}") for i in range(2)]
            h_half = [stp.tile([128, 128], b16, name=f"h{i}") for i in range(2)]
            for x in (*c_half, *h_half):
                nc.vector.memset(x[:], 0.0)

            enc_t, feats_t, xg_t, hout_t = {}, {}, {}, {}

            def load_enc(cc):
                t_ = encp.tile([128, KDE * NCOL], b16, tag="enc", name="enc")
                nc.scalar.dma_start(t_[:], encT_d[:, :, cc * NCOL:(cc + 1) * NCOL])
                enc_t[cc] = t_

            def g1_group(cc, m):
                # feats.T rows [128m, 128m+128) for chunk cc
                if m == 0:
                    feats_t[cc] = fp.tile([128, KDF * NCOL], b16, tag="feats", name="feats")
                ps = ps1.tile([128, NCOL], f32, tag="g1")
                e = enc_t[cc]
                for k in range(KDE):
                    nc.tensor.matmul(
                        ps[:],
                        wproj_sb[:, k * DF + m * 128: k * DF + m * 128 + 128],
                        e[:, k * NCOL:(k + 1) * NCOL],
                        start=(k == 0), stop=(k == KDE - 1),
                    )
                with tc.high_priority(-200):
                    nc.vector.tensor_copy(
                        feats_t[cc][:, m * NCOL:(m + 1) * NCOL], ps[:])

            def g2_group(cc, m):
                # XG.T rows [128m, 128m+128) for chunk cc (+ bias fold-in)
                if m == 0:
                    xg_t[cc] = xgp.tile([128, MT * NCOL], b16, tag="xg", name="xg")
                ps = ps2.tile([128, NCOL], f32, tag="g2")
                f_ = feats_t[cc]
                for k in range(KDF):
                    nc.tensor.matmul(
                        ps[:],
                        wx_sb[:, k * G4 + m * 128: k * G4 + m * 128 + 128],
                        f_[:, k * NCOL:(k + 1) * NCOL],
                        start=(k == 0), stop=(k == KDF - 1),
                    )
                with tc.high_priority(-200):
                    nc.vector.scalar_tensor_tensor(
                        xg_t[cc][:, m * NCOL:(m + 1) * NCOL],
                        ps[:], 1.0,
                        biasr_sb[:, m * NCOL:(m + 1) * NCOL],
                        op0=ALU.mult, op1=ALU.add,
                    )

            def rec_bank(t, gps, bank):
                # inject XG (clears the bank), then accumulate W_hh.T @ h.T
                cc, ts = t // CH, t % CH
                xg3 = xg_t[cc][:].rearrange("p (m n) -> p m n", m=MT)
                nc.tensor.matmul(
                    gps[:],
                    ident_sb[:],
                    xg3[:, bank * 8:(bank + 1) * 8, ts * BL:(ts + 1) * BL],
                    start=True, stop=False, skip_group_check=True,
                )
                for k in range(KH):
                    for m in range(bank * 8, bank * 8 + 8):
                        nc.tensor.matmul(
                            gps[:, (m % 8) * BL:(m % 8 + 1) * BL],
                            whh_sb[:, k * G4 + m * 128: k * G4 + m * 128 + 128],
                            h_half[k // 2][:, (k % 2) * BL:(k % 2 + 1) * BL],
                            start=False,
                            stop=(k == KH - 1 and m % 8 == 7),
                            skip_group_check=True,
                        )

            def eltwise_half(t, gps, hf):
                cc, ts = t // CH, t % CH
                ho3 = hout_t[cc][:].rearrange("p (j n) -> p j n", j=KH)
                act = ewp.tile([128, 512], f32, tag="act", name="act")
                nc.scalar.activation(act[:, 0:384], gps[:, 0:384], AF.Sigmoid)
                nc.scalar.activation(act[:, 384:512], gps[:, 384:512], AF.Tanh)
                t2 = ewp.tile([128, 128], f32, tag="t2", name="t2")
                cs = c_half[hf]
                nc.vector.tensor_mul(t2[:], act[:, 128:256], cs[:])
                t1 = ewp.tile([128, 128], f32, tag="t1", name="t1")
                nc.vector.tensor_mul(t1[:], act[:, 0:128], act[:, 384:512])
                nc.vector.tensor_add(cs[:], t1[:], t2[:])
                tc_ = ewp.tile([128, 128], f32, tag="tc", name="tc")
                nc.scalar.activation(tc_[:], cs[:], AF.Tanh)
                # h.T state first (bf16, feeds next-step matmul = critical path)
                nc.vector.tensor_mul(h_half[hf][:], act[:, 256:384], tc_[:])
                # h -> f32 output staging, off the critical path (GpSimd lags,
                # so the final chunk uses DVE to not delay the last DMA)
                eng = nc.vector if cc == NCH - 1 else nc.gpsimd
                so2 = act[:, 256:384].rearrange("p (j n) -> p j n", j=2)
                tc2 = tc_[:].rearrange("p (j n) -> p j n", j=2)
                eng.tensor_mul(
                    ho3[:, 2 * hf:2 * hf + 2, ts * BL:(ts + 1) * BL], so2, tc2)

            # ---- prologue: chunks 0/1 feats, chunk 0 XG ----
            load_enc(0)
            load_enc(1)
            nc.sync.dma_start(wx_sb[:], wx_d[:])
            nc.sync.dma_start(biasr_sb[:], biasr_d[:])
            nc.sync.dma_start(whh_sb[:], whh_d[:])
            for m in range(KDF):
                g1_group(0, m)
            for m in range(KDF):
                g1_group(1, m)
            for m in range(MT):
                g2_group(0, m)

            # ---- main loop ----
            for t in range(T):
                cc, ts = t // CH, t % CH
                if ts == 0:
                    hout_t[cc] = hop.tile([128, KH * CH * BL], f32, tag="hout", name="hout")
                # separate psum tiles per half so eltwise(0) doesn't wait bank 1
                gpsA = psg.tile([128, 512], f32, tag="gatesA", name="gatesA")
                gpsB = psg.tile([128, 512], f32, tag="gatesB", name="gatesB")
                rec_bank(t, gpsA, 0)
                rec_bank(t, gpsB, 1)
                eltwise_half(t, gpsA, 0)
                eltwise_half(t, gpsB, 1)
                # GEMM quota for future chunks fills PE while eltwise runs
                if cc + 2 < NCH:
                    if ts == 0:
                        load_enc(cc + 2)
                    if ts % 2 == 0:
                        g1_group(cc + 2, ts // 2)
                if cc + 1 < NCH:
                    g2_group(cc + 1, 2 * ts)
                    g2_group(cc + 1, 2 * ts + 1)
                if ts == CH - 1:
                    nc.scalar.dma_start(
                        hT_d[:, :, cc * CH:(cc + 1) * CH, :], hout_t[cc][:])

    nc.compile()
    return nc


def _get_compiled():
    global _COMPILED
    if _COMPILED is None:
        _COMPILED = _build()
    return _COMPILED


def _prep_maps(outputs_encoder, start_ids, W_proj, b_proj, embed_table,
               W_ih, W_hh, b_ih, b_hh):
    outputs_encoder = np.asarray(outputs_encoder, np.float32)
    start_ids = np.asarray(start_ids)
    W_proj = np.asarray(W_proj, np.float32)
    b_proj = np.asarray(b_proj, np.float32)
    embed_table = np.asarray(embed_table, np.float32)
    W_ih = np.asarray(W_ih, np.float32)
    W_hh = np.asarray(W_hh, np.float32)
    b_ih = np.asarray(b_ih, np.float32)
    b_hh = np.asarray(b_hh, np.float32)

    # gate-row permutation: [i0 f0 o0 g0 i1 f1 o1 g1] (torch order i,f,g,o)
    perm = []
    for half in range(2):
        for g0 in (0, 1, 3, 2):
            base = g0 * H + half * 256
            perm.extend(range(base, base + 256))
    perm = np.asarray(perm)

    W_ih_p = W_ih[perm]
    W_hh_p = W_hh[perm]
    bvec_p = (b_ih + b_hh)[perm]
    W_x = W_ih_p[:, :DF]
    W_w = W_ih_p[:, DF:]

    word = embed_table[start_ids]                       # [B, DW]
    # full (t,b)-constant gate bias: word part + b_ih + b_hh + b_proj @ W_x.T
    biasw = word @ W_w.T + bvec_p[None, :] + (b_proj @ W_x.T)[None, :]  # [B, G4]

    del bvec_p
    wproj_arr = np.ascontiguousarray(
        W_proj.reshape(KDE, 128, DF).transpose(1, 0, 2)).astype(BF16)
    wx_arr = np.ascontiguousarray(
        W_x.T.reshape(KDF, 128, G4).transpose(1, 0, 2)).astype(BF16)
    whh_arr = np.ascontiguousarray(
        W_hh_p.T.reshape(KH, 128, G4).transpose(1, 0, 2)).astype(BF16)
    in_maps = []
    for c in range(NCORES):
        bsl = slice(c * BL, (c + 1) * BL)
        enc_c = outputs_encoder[bsl]                    # [64, 80, 1024]
        encT = np.ascontiguousarray(
            enc_c.transpose(2, 1, 0)                    # [1024, 80, 64]
            .reshape(KDE, 128, T * BL)
            .transpose(1, 0, 2)).astype(BF16)           # [128, 8, 5120]
        # gate-major bias [2048, 64] -> [128, m(16), 64], repeated CH x in t
        bgm = biasw[bsl].T.reshape(MT, 128, BL).transpose(1, 0, 2)  # [128,16,64]
        biasr = np.broadcast_to(bgm[:, :, None, :], (128, MT, CH, BL))
        biasr = np.ascontiguousarray(biasr).reshape(128, MT * NCOL).astype(BF16)
        in_maps.append({
            "ident": np.eye(128, dtype=np.float32).astype(BF16),
            "encT": encT,
            "wproj": wproj_arr,
            "wx": wx_arr,
            "whh": whh_arr,
            "biasr": biasr,
        })
    return in_maps


def run_on_device(in_maps, trace=False):
    from concourse.bass_utils import run_bass_kernel_spmd
    nc = _get_compiled()
    return run_bass_kernel_spmd(
        nc, in_maps, core_ids=list(range(NCORES)), trace=trace)


def kernel(**inputs):
    in_maps = _prep_maps(**inputs)
    res = run_on_device(in_maps)
    out = np.empty((B, T, H), np.float32)
    for c in range(NCORES):
        hT = res.results[c]["hT"]                       # [128, 4, 80, 64]
        out[c * BL:(c + 1) * BL] = (
            hT.transpose(3, 2, 1, 0).reshape(BL, T, H))
    return out


# revision 22
# speedup vs baseline: 1.0536x; 1.0536x over previous
"""Trainium2 Bass kernel for nn_Decoder (LSTM decoder over encoder features).

Math (per reference):
    feats = enc @ W_proj + b_proj            [B, T, DF]
    word  = embed[start_ids]                 [B, DW]   (constant per step)
    x_t   = concat(feats_t, word)
    gates = x_t @ W_ih.T + h @ W_hh.T + b    -> LSTM cell -> h_t (output)

Kernel strategy (8 cores, data-parallel over batch, B_local = 64):
  Everything on-device is kept "gate-major" (transposed: [dims, batch]) so no
  on-device transposes are ever needed:
    GEMM1: feats.T[DF, (t,b)] = W_proj(lhsT) @ enc.T(rhs)       streamed N=512
    GEMM2: XG.T[4H, (t,b)]    = W_x.T(lhsT) @ feats.T(rhs)
            + bias via a selector matmul (bias_pad(lhsT) @ S(rhs))
    rec_t: gates.T[4H, b]    += W_hh.T(lhsT) @ h.T(rhs)         into PSUM
    eltwise: sigmoid/tanh on ACT, mul/add on DVE, c kept fp32.
  The word-embedding gather, the W_proj/W_ih split, the gate-row permutation
  and all layout transposes are host-side precomputation (numpy only).

  Gate rows are permuted to [i0 f0 o0 g0 i1 f1 o1 g1] (256-row blocks) so each
  half-step's sigmoid operand (i,f,o) is one contiguous ACT instruction.

Output is written transposed (hT[p, j, t, b]) and untransposed on the host.
"""

import numpy as np
import ml_dtypes

BF16 = ml_dtypes.bfloat16

# Problem dims (hardcoded per spec)
NCORES = 8
B, T, DE, DF, DW, H, V = 512, 80, 1024, 512, 512, 512, 10000
G4 = 4 * H                      # 2048 gate rows
BL = B // NCORES                # 64 batch per core
CH = 8                          # timesteps per GEMM chunk
NCH = T // CH                   # 10 chunks
KDE = DE // 128                 # 8  contraction chunks for GEMM1
KDF = DF // 128                 # 4  contraction chunks for GEMM2
KH = H // 128                   # 4  contraction chunks for recurrence
MT = G4 // 128                  # 16 gate-row tiles
NCOL = CH * BL                  # 512 (t,b) columns per chunk

_COMPILED = None


def _build():
    import concourse.bacc as bacc
    import concourse.tile as tile
    import concourse.mybir as mybir
    import concourse.bass as bass

    dt = mybir.dt
    f32, b16 = dt.float32, dt.bfloat16
    AF = mybir.ActivationFunctionType
    ALU = mybir.AluOpType

    nc = bacc.Bacc("TRN2", target_bir_lowering=False, debug=False,
                   num_devices=NCORES)

    ident_d = nc.dram_tensor("ident", [128, 128], b16, kind="ExternalInput")
    encT_d = nc.dram_tensor("encT", [128, KDE, T * BL], b16, kind="ExternalInput")
    wproj_d = nc.dram_tensor("wproj", [128, KDE, DF], b16, kind="ExternalInput")
    wx_d = nc.dram_tensor("wx", [128, KDF, G4], b16, kind="ExternalInput")
    whh_d = nc.dram_tensor("whh", [128, KH, G4], b16, kind="ExternalInput")
    biasr_d = nc.dram_tensor("biasr", [128, MT * NCOL], b16, kind="ExternalInput")
    hT_d = nc.dram_tensor("hT", [128, KH, T, BL], f32, kind="ExternalOutput")

    with tile.TileContext(nc) as tc:
        with (
            tc.tile_pool(name="wpool", bufs=1) as wp,
            tc.tile_pool(name="encp", bufs=3) as encp,
            tc.tile_pool(name="featsp", bufs=3) as fp,
            tc.tile_pool(name="xgp", bufs=2) as xgp,
            tc.tile_pool(name="statep", bufs=1) as stp,
            tc.tile_pool(name="ewp", bufs=10) as ewp,
            tc.tile_pool(name="houtp", bufs=2) as hop,
            tc.tile_pool(name="psg", bufs=2, space=bass.MemorySpace.PSUM) as psg,
            tc.tile_pool(name="ps1", bufs=1, space=bass.MemorySpace.PSUM) as ps1,
            tc.tile_pool(name="ps2", bufs=3, space=bass.MemorySpace.PSUM) as ps2,
        ):
            # ---- persistent weights ----
            wproj_sb = wp.tile([128, KDE * DF], b16)
            wx_sb = wp.tile([128, KDF * G4], b16)
            whh_sb = wp.tile([128, KH * G4], b16)
            biasr_sb = wp.tile([128, MT * NCOL], b16)
            ident_sb = wp.tile([128, 128], b16)
            nc.sync.dma_start(ident_sb[:], ident_d[:])
            nc.sync.dma_start(wproj_sb[:], wproj_d[:])

            # warmup: keep PE busy (and HAM un-throttled) while DMAs land
            warm_ps = ps2.tile([128, 128], f32, tag="g2", name="warm")
            for _ in range(40):
                nc.tensor.matmul(warm_ps[:], ident_sb[:], ident_sb[:],
                                 start=True, stop=True)

            # ---- state (split per half so Tile deps don't serialize halves) ----
            c_half = [stp.tile([128, 128], f32, name=f"c{i}") for i in range(2)]
            h_half = [stp.tile([128, 128], b16, name=f"h{i}") for i in range(2)]
            for x in (*c_half, *h_half):
                nc.vector.memset(x[:], 0.0)

            enc_t, feats_t, xg_t, hout_t = {}, {}, {}, {}

            def load_enc(cc):
                t_ = encp.tile([128, KDE * NCOL], b16, tag="enc", name="enc")
                nc.sync.dma_start(t_[:], encT_d[:, :, cc * NCOL:(cc + 1) * NCOL])
                enc_t[cc] = t_

            def g1_group(cc, m):
                # feats.T rows [128m, 128m+128) for chunk cc
                if m == 0:
                    feats_t[cc] = fp.tile([128, KDF * NCOL], b16, tag="feats", name="feats")
                ps = ps1.tile([128, NCOL], f32, tag="g1")
                e = enc_t[cc]
                for k in range(KDE):
                    nc.tensor.matmul(
                        ps[:],
                        wproj_sb[:, k * DF + m * 128: k * DF + m * 128 + 128],
                        e[:, k * NCOL:(k + 1) * NCOL],
                        start=(k == 0), stop=(k == KDE - 1),
                    )
                with tc.high_priority(-200):
                    nc.vector.tensor_copy(
                        feats_t[cc][:, m * NCOL:(m + 1) * NCOL], ps[:])

            def g2_group(cc, m):
                # XG.T rows [128m, 128m+128) for chunk cc (+ bias fold-in)
                if m == 0:
                    xg_t[cc] = xgp.tile([128, MT * NCOL], b16, tag="xg", name="xg")
                ps = ps2.tile([128, NCOL], f32, tag="g2")
                f_ = feats_t[cc]
                for k in range(KDF):
                    nc.tensor.matmul(
                        ps[:],
                        wx_sb[:, k * G4 + m * 128: k * G4 + m * 128 + 128],
                        f_[:, k * NCOL:(k + 1) * NCOL],
                        start=(k == 0), stop=(k == KDF - 1),
                    )
                with tc.high_priority(-200):
                    nc.vector.scalar_tensor_tensor(
                        xg_t[cc][:, m * NCOL:(m + 1) * NCOL],
                        ps[:], 1.0,
                        biasr_sb[:, m * NCOL:(m + 1) * NCOL],
                        op0=ALU.mult, op1=ALU.add,
                    )

            def rec_bank(t, gps, bank):
                # inject XG (clears the bank), then accumulate W_hh.T @ h.T
                cc, ts = t // CH, t % CH
                xg3 = xg_t[cc][:].rearrange("p (m n) -> p m n", m=MT)
                nc.tensor.matmul(
                    gps[:],
                    ident_sb[:],
                    xg3[:, bank * 8:(bank + 1) * 8, ts * BL:(ts + 1) * BL],
                    start=True, stop=False, skip_group_check=True,
                )
                for k in range(KH):
                    for m in range(bank * 8, bank * 8 + 8):
                        nc.tensor.matmul(
                            gps[:, (m % 8) * BL:(m % 8 + 1) * BL],
                            whh_sb[:, k * G4 + m * 128: k * G4 + m * 128 + 128],
                            h_half[k // 2][:, (k % 2) * BL:(k % 2 + 1) * BL],
                            start=False,
                            stop=(k == KH - 1 and m % 8 == 7),
                            skip_group_check=True,
                        )

            def eltwise_half(t, gps, hf):
                cc, ts = t // CH, t % CH
                ho3 = hout_t[cc][:].rearrange("p (j n) -> p j n", j=KH)
                act = ewp.tile([128, 512], f32, tag="act", name="act")
                nc.scalar.activation(act[:, 0:384], gps[:, 0:384], AF.Sigmoid)
                nc.scalar.activation(act[:, 384:512], gps[:, 384:512], AF.Tanh)
                t2 = ewp.tile([128, 128], f32, tag="t2", name="t2")
                cs = c_half[hf]
                nc.vector.tensor_mul(t2[:], act[:, 128:256], cs[:])
                t1 = ewp.tile([128, 128], f32, tag="t1", name="t1")
                nc.vector.tensor_mul(t1[:], act[:, 0:128], act[:, 384:512])
                nc.vector.tensor_add(cs[:], t1[:], t2[:])
                tc_ = ewp.tile([128, 128], f32, tag="tc", name="tc")
                nc.scalar.activation(tc_[:], cs[:], AF.Tanh)
                # h.T state first (bf16, feeds next-step matmul = critical path)
                nc.vector.tensor_mul(h_half[hf][:], act[:, 256:384], tc_[:])
                # h -> f32 output staging, off the critical path (GpSimd lags,
                # so the final chunk uses DVE to not delay the last DMA)
                eng = nc.vector if cc == NCH - 1 else nc.gpsimd
                so2 = act[:, 256:384].rearrange("p (j n) -> p j n", j=2)
                tc2 = tc_[:].rearrange("p (j n) -> p j n", j=2)
                eng.tensor_mul(
                    ho3[:, 2 * hf:2 * hf + 2, ts * BL:(ts + 1) * BL], so2, tc2)

            # ---- prologue: chunks 0/1 feats, chunk 0 XG ----
            load_enc(0)
            load_enc(1)
            nc.sync.dma_start(wx_sb[:], wx_d[:])
            nc.sync.dma_start(biasr_sb[:], biasr_d[:])
            nc.sync.dma_start(whh_sb[:], whh_d[:])
            for m in range(KDF):
                g1_group(0, m)
            for m in range(KDF):
                g1_group(1, m)
            for m in range(MT):
                g2_group(0, m)

            # ---- main loop ----
            for t in range(T):
                cc, ts = t // CH, t % CH
                if ts == 0:
                    hout_t[cc] = hop.tile([128, KH * CH * BL], f32, tag="hout", name="hout")
                # separate psum tiles per half so eltwise(0) doesn't wait bank 1
                gpsA = psg.tile([128, 512], f32, tag="gatesA", name="gatesA")
                gpsB = psg.tile([128, 512], f32, tag="gatesB", name="gatesB")
                rec_bank(t, gpsA, 0)
                rec_bank(t, gpsB, 1)
                eltwise_half(t, gpsA, 0)
                eltwise_half(t, gpsB, 1)
                # GEMM quota for future chunks fills PE while eltwise runs
                if cc + 2 < NCH:
                    if ts == 0:
                        load_enc(cc + 2)
                    if ts % 2 == 0:
                        g1_group(cc + 2, ts // 2)
                if cc + 1 < NCH:
                    g2_group(cc + 1, 2 * ts)
                    g2_group(cc + 1, 2 * ts + 1)
                if ts == CH - 1:
                    nc.sync.dma_start(
                        hT_d[:, :, cc * CH:(cc + 1) * CH, :], hout_t[cc][:])

    nc.compile()
    return nc


def _get_compiled():
    global _COMPILED
    if _COMPILED is None:
        _COMPILED = _build()
    return _COMPILED


def _prep_maps(outputs_encoder, start_ids, W_proj, b_proj, embed_table,
               W_ih, W_hh, b_ih, b_hh):
    outputs_encoder = np.asarray(outputs_encoder, np.float32)
    start_ids = np.asarray(start_ids)
    W_proj = np.asarray(W_proj, np.float32)
    b_proj = np.asarray(b_proj, np.float32)
    embed_table = np.asarray(embed_table, np.float32)
    W_ih = np.asarray(W_ih, np.float32)
    W_hh = np.asarray(W_hh, np.float32)
    b_ih = np.asarray(b_ih, np.float32)
    b_hh = np.asarray(b_hh, np.float32)

    # gate-row permutation: [i0 f0 o0 g0 i1 f1 o1 g1] (torch order i,f,g,o)
    perm = []
    for half in range(2):
        for g0 in (0, 1, 3, 2):
            base = g0 * H + half * 256
            perm.extend(range(base, base + 256))
    perm = np.asarray(perm)

    W_ih_p = W_ih[perm]
    W_hh_p = W_hh[perm]
    bvec_p = (b_ih + b_hh)[perm]
    W_x = W_ih_p[:, :DF]
    W_w = W_ih_p[:, DF:]

    word = embed_table[start_ids]                       # [B, DW]
    # full (t,b)-constant gate bias: word part + b_ih + b_hh + b_proj @ W_x.T
    biasw = word @ W_w.T + bvec_p[None, :] + (b_proj @ W_x.T)[None, :]  # [B, G4]

    del bvec_p
    wproj_arr = np.ascontiguousarray(
        W_proj.reshape(KDE, 128, DF).transpose(1, 0, 2)).astype(BF16)
    wx_arr = np.ascontiguousarray(
        W_x.T.reshape(KDF, 128, G4).transpose(1, 0, 2)).astype(BF16)
    whh_arr = np.ascontiguousarray(
        W_hh_p.T.reshape(KH, 128, G4).transpose(1, 0, 2)).astype(BF16)
    in_maps = []
    for c in range(NCORES):
        bsl = slice(c * BL, (c + 1) * BL)
        enc_c = outputs_encoder[bsl]                    # [64, 80, 1024]
        encT = np.ascontiguousarray(
            enc_c.transpose(2, 1, 0)                    # [1024, 80, 64]
            .reshape(KDE, 128, T * BL)
            .transpose(1, 0, 2)).astype(BF16)           # [128, 8, 5120]
        # gate-major bias [2048, 64] -> [128, m(16), 64], repeated CH x in t
        bgm = biasw[bsl].T.reshape(MT, 128, BL).transpose(1, 0, 2)  # [128,16,64]
        biasr = np.broadcast_to(bgm[:, :, None, :], (128, MT, CH, BL))
        biasr = np.ascontiguousarray(biasr).reshape(128, MT * NCOL).astype(BF16)
        in_maps.append({
            "ident": np.eye(128, dtype=np.float32).astype(BF16),
            "encT": encT,
            "wproj": wproj_arr,
            "wx": wx_arr,
            "whh": whh_arr,
            "biasr": biasr,
        })
    return in_maps


def run_on_device(in_maps, trace=False):
    from concourse.bass_utils import run_bass_kernel_spmd
    nc = _get_compiled()
    return run_bass_kernel_spmd(
        nc, in_maps, core_ids=list(range(NCORES)), trace=trace)


def kernel(**inputs):
    in_maps = _prep_maps(**inputs)
    res = run_on_device(in_maps)
    out = np.empty((B, T, H), np.float32)
    for c in range(NCORES):
        hT = res.results[c]["hT"]                       # [128, 4, 80, 64]
        out[c * BL:(c + 1) * BL] = (
            hT.transpose(3, 2, 1, 0).reshape(BL, T, H))
    return out


# revision 23
# speedup vs baseline: 1.0613x; 1.0073x over previous
"""Trainium2 Bass kernel for nn_Decoder (LSTM decoder over encoder features).

Math (per reference):
    feats = enc @ W_proj + b_proj            [B, T, DF]
    word  = embed[start_ids]                 [B, DW]   (constant per step)
    x_t   = concat(feats_t, word)
    gates = x_t @ W_ih.T + h @ W_hh.T + b    -> LSTM cell -> h_t (output)

Kernel strategy (8 cores, data-parallel over batch, B_local = 64):
  Everything on-device is kept "gate-major" (transposed: [dims, batch]) so no
  on-device transposes are ever needed:
    GEMM1: feats.T[DF, (t,b)] = W_proj(lhsT) @ enc.T(rhs)       streamed N=512
    GEMM2: XG.T[4H, (t,b)]    = W_x.T(lhsT) @ feats.T(rhs)
            + bias via a selector matmul (bias_pad(lhsT) @ S(rhs))
    rec_t: gates.T[4H, b]    += W_hh.T(lhsT) @ h.T(rhs)         into PSUM
    eltwise: sigmoid/tanh on ACT, mul/add on DVE, c kept fp32.
  The word-embedding gather, the W_proj/W_ih split, the gate-row permutation
  and all layout transposes are host-side precomputation (numpy only).

  Gate rows are permuted to [i0 f0 o0 g0 i1 f1 o1 g1] (256-row blocks) so each
  half-step's sigmoid operand (i,f,o) is one contiguous ACT instruction.

Output is written transposed (hT[p, j, t, b]) and untransposed on the host.
"""

import numpy as np
import ml_dtypes

BF16 = ml_dtypes.bfloat16

# Problem dims (hardcoded per spec)
NCORES = 8
B, T, DE, DF, DW, H, V = 512, 80, 1024, 512, 512, 512, 10000
G4 = 4 * H                      # 2048 gate rows
BL = B // NCORES                # 64 batch per core
CH = 4                          # timesteps per GEMM chunk
NCH = T // CH                   # 10 chunks
KDE = DE // 128                 # 8  contraction chunks for GEMM1
KDF = DF // 128                 # 4  contraction chunks for GEMM2
KH = H // 128                   # 4  contraction chunks for recurrence
MT = G4 // 128                  # 16 gate-row tiles
NCOL = CH * BL                  # 512 (t,b) columns per chunk

_COMPILED = None


def _build():
    import concourse.bacc as bacc
    import concourse.tile as tile
    import concourse.mybir as mybir
    import concourse.bass as bass

    dt = mybir.dt
    f32, b16 = dt.float32, dt.bfloat16
    AF = mybir.ActivationFunctionType
    ALU = mybir.AluOpType

    nc = bacc.Bacc("TRN2", target_bir_lowering=False, debug=False,
                   num_devices=NCORES)

    ident_d = nc.dram_tensor("ident", [128, 128], b16, kind="ExternalInput")
    encT_d = nc.dram_tensor("encT", [128, KDE, T * BL], b16, kind="ExternalInput")
    wproj_d = nc.dram_tensor("wproj", [128, KDE, DF], b16, kind="ExternalInput")
    wx_d = nc.dram_tensor("wx", [128, KDF, G4], b16, kind="ExternalInput")
    whh_d = nc.dram_tensor("whh", [128, KH, G4], b16, kind="ExternalInput")
    biasr_d = nc.dram_tensor("biasr", [128, MT * NCOL], b16, kind="ExternalInput")
    hT_d = nc.dram_tensor("hT", [128, KH, T, BL], f32, kind="ExternalOutput")

    with tile.TileContext(nc) as tc:
        with (
            tc.tile_pool(name="wpool", bufs=1) as wp,
            tc.tile_pool(name="encp", bufs=3) as encp,
            tc.tile_pool(name="featsp", bufs=3) as fp,
            tc.tile_pool(name="xgp", bufs=2) as xgp,
            tc.tile_pool(name="statep", bufs=1) as stp,
            tc.tile_pool(name="ewp", bufs=10) as ewp,
            tc.tile_pool(name="houtp", bufs=2) as hop,
            tc.tile_pool(name="psg", bufs=2, space=bass.MemorySpace.PSUM) as psg,
            tc.tile_pool(name="ps1", bufs=1, space=bass.MemorySpace.PSUM) as ps1,
            tc.tile_pool(name="ps2", bufs=3, space=bass.MemorySpace.PSUM) as ps2,
        ):
            # ---- persistent weights ----
            wproj_sb = wp.tile([128, KDE * DF], b16)
            wx_sb = wp.tile([128, KDF * G4], b16)
            whh_sb = wp.tile([128, KH * G4], b16)
            biasr_sb = wp.tile([128, MT * NCOL], b16)
            ident_sb = wp.tile([128, 128], b16)
            nc.sync.dma_start(ident_sb[:], ident_d[:])
            nc.sync.dma_start(wproj_sb[:], wproj_d[:])

            # warmup: keep PE busy (and HAM un-throttled) while DMAs land
            warm_ps = ps2.tile([128, 128], f32, tag="g2", name="warm")
            for _ in range(40):
                nc.tensor.matmul(warm_ps[:], ident_sb[:], ident_sb[:],
                                 start=True, stop=True)

            # ---- state (split per half so Tile deps don't serialize halves) ----
            c_half = [stp.tile([128, 128], f32, name=f"c{i}") for i in range(2)]
            h_half = [stp.tile([128, 128], b16, name=f"h{i}") for i in range(2)]
            for x in (*c_half, *h_half):
                nc.vector.memset(x[:], 0.0)

            enc_t, feats_t, xg_t, hout_t = {}, {}, {}, {}

            def load_enc(cc):
                t_ = encp.tile([128, KDE * NCOL], b16, tag="enc", name="enc")
                nc.sync.dma_start(t_[:], encT_d[:, :, cc * NCOL:(cc + 1) * NCOL])
                enc_t[cc] = t_

            def g1_group(cc, m):
                # feats.T rows [128m, 128m+128) for chunk cc
                if m == 0:
                    feats_t[cc] = fp.tile([128, KDF * NCOL], b16, tag="feats", name="feats")
                ps = ps1.tile([128, NCOL], f32, tag="g1")
                e = enc_t[cc]
                for k in range(KDE):
                    nc.tensor.matmul(
                        ps[:],
                        wproj_sb[:, k * DF + m * 128: k * DF + m * 128 + 128],
                        e[:, k * NCOL:(k + 1) * NCOL],
                        start=(k == 0), stop=(k == KDE - 1),
                    )
                with tc.high_priority(-200):
                    nc.vector.tensor_copy(
                        feats_t[cc][:, m * NCOL:(m + 1) * NCOL], ps[:])

            def g2_group(cc, m):
                # XG.T rows [128m, 128m+128) for chunk cc (+ bias fold-in)
                if m == 0:
                    xg_t[cc] = xgp.tile([128, MT * NCOL], b16, tag="xg", name="xg")
                ps = ps2.tile([128, NCOL], f32, tag="g2")
                f_ = feats_t[cc]
                for k in range(KDF):
                    nc.tensor.matmul(
                        ps[:],
                        wx_sb[:, k * G4 + m * 128: k * G4 + m * 128 + 128],
                        f_[:, k * NCOL:(k + 1) * NCOL],
                        start=(k == 0), stop=(k == KDF - 1),
                    )
                with tc.high_priority(-200):
                    nc.vector.scalar_tensor_tensor(
                        xg_t[cc][:, m * NCOL:(m + 1) * NCOL],
                        ps[:], 1.0,
                        biasr_sb[:, m * NCOL:(m + 1) * NCOL],
                        op0=ALU.mult, op1=ALU.add,
                    )

            def rec_bank(t, gps, bank):
                # inject XG (clears the bank), then accumulate W_hh.T @ h.T
                cc, ts = t // CH, t % CH
                xg3 = xg_t[cc][:].rearrange("p (m n) -> p m n", m=MT)
                nc.tensor.matmul(
                    gps[:],
                    ident_sb[:],
                    xg3[:, bank * 8:(bank + 1) * 8, ts * BL:(ts + 1) * BL],
                    start=True, stop=False, skip_group_check=True,
                )
                for k in range(KH):
                    for m in range(bank * 8, bank * 8 + 8):
                        nc.tensor.matmul(
                            gps[:, (m % 8) * BL:(m % 8 + 1) * BL],
                            whh_sb[:, k * G4 + m * 128: k * G4 + m * 128 + 128],
                            h_half[k // 2][:, (k % 2) * BL:(k % 2 + 1) * BL],
                            start=False,
                            stop=(k == KH - 1 and m % 8 == 7),
                            skip_group_check=True,
                        )

            def eltwise_half(t, gps, hf):
                cc, ts = t // CH, t % CH
                ho3 = hout_t[cc][:].rearrange("p (j n) -> p j n", j=KH)
                act = ewp.tile([128, 512], f32, tag="act", name="act")
                nc.scalar.activation(act[:, 0:384], gps[:, 0:384], AF.Sigmoid)
                nc.scalar.activation(act[:, 384:512], gps[:, 384:512], AF.Tanh)
                t2 = ewp.tile([128, 128], f32, tag="t2", name="t2")
                cs = c_half[hf]
                nc.vector.tensor_mul(t2[:], act[:, 128:256], cs[:])
                t1 = ewp.tile([128, 128], f32, tag="t1", name="t1")
                nc.vector.tensor_mul(t1[:], act[:, 0:128], act[:, 384:512])
                nc.vector.tensor_add(cs[:], t1[:], t2[:])
                tc_ = ewp.tile([128, 128], f32, tag="tc", name="tc")
                nc.scalar.activation(tc_[:], cs[:], AF.Tanh)
                # h.T state first (bf16, feeds next-step matmul = critical path)
                nc.vector.tensor_mul(h_half[hf][:], act[:, 256:384], tc_[:])
                # h -> f32 output staging, off the critical path (GpSimd lags,
                # so the final chunk uses DVE to not delay the last DMA)
                eng = nc.vector if cc == NCH - 1 else nc.gpsimd
                so2 = act[:, 256:384].rearrange("p (j n) -> p j n", j=2)
                tc2 = tc_[:].rearrange("p (j n) -> p j n", j=2)
                eng.tensor_mul(
                    ho3[:, 2 * hf:2 * hf + 2, ts * BL:(ts + 1) * BL], so2, tc2)

            # ---- prologue: chunks 0/1 feats, chunk 0 XG ----
            load_enc(0)
            load_enc(1)
            nc.sync.dma_start(wx_sb[:], wx_d[:])
            nc.sync.dma_start(biasr_sb[:], biasr_d[:])
            nc.sync.dma_start(whh_sb[:], whh_d[:])
            for m in range(KDF):
                g1_group(0, m)
            for m in range(KDF):
                g1_group(1, m)
            for m in range(MT):
                g2_group(0, m)

            # ---- main loop ----
            for t in range(T):
                cc, ts = t // CH, t % CH
                if ts == 0:
                    hout_t[cc] = hop.tile([128, KH * CH * BL], f32, tag="hout", name="hout")
                # separate psum tiles per half so eltwise(0) doesn't wait bank 1
                gpsA = psg.tile([128, 512], f32, tag="gatesA", name="gatesA")
                gpsB = psg.tile([128, 512], f32, tag="gatesB", name="gatesB")
                rec_bank(t, gpsA, 0)
                rec_bank(t, gpsB, 1)
                eltwise_half(t, gpsA, 0)
                eltwise_half(t, gpsB, 1)
                # GEMM quota for future chunks fills PE while eltwise runs
                if cc + 2 < NCH:
                    if ts == 0:
                        load_enc(cc + 2)
                    g1_group(cc + 2, ts)
                if cc + 1 < NCH:
                    for q in range(4):
                        g2_group(cc + 1, 4 * ts + q)
                if ts == CH - 1:
                    nc.sync.dma_start(
                        hT_d[:, :, cc * CH:(cc + 1) * CH, :], hout_t[cc][:])

    nc.compile()
    return nc


def _get_compiled():
    global _COMPILED
    if _COMPILED is None:
        _COMPILED = _build()
    return _COMPILED


def _prep_maps(outputs_encoder, start_ids, W_proj, b_proj, embed_table,
               W_ih, W_hh, b_ih, b_hh):
    outputs_encoder = np.asarray(outputs_encoder, np.float32)
    start_ids = np.asarray(start_ids)
    W_proj = np.asarray(W_proj, np.float32)
    b_proj = np.asarray(b_proj, np.float32)
    embed_table = np.asarray(embed_table, np.float32)
    W_ih = np.asarray(W_ih, np.float32)
    W_hh = np.asarray(W_hh, np.float32)
    b_ih = np.asarray(b_ih, np.float32)
    b_hh = np.asarray(b_hh, np.float32)

    # gate-row permutation: [i0 f0 o0 g0 i1 f1 o1 g1] (torch order i,f,g,o)
    perm = []
    for half in range(2):
        for g0 in (0, 1, 3, 2):
            base = g0 * H + half * 256
            perm.extend(range(base, base + 256))
    perm = np.asarray(perm)

    W_ih_p = W_ih[perm]
    W_hh_p = W_hh[perm]
    bvec_p = (b_ih + b_hh)[perm]
    W_x = W_ih_p[:, :DF]
    W_w = W_ih_p[:, DF:]

    word = embed_table[start_ids]                       # [B, DW]
    # full (t,b)-constant gate bias: word part + b_ih + b_hh + b_proj @ W_x.T
    biasw = word @ W_w.T + bvec_p[None, :] + (b_proj @ W_x.T)[None, :]  # [B, G4]

    del bvec_p
    wproj_arr = np.ascontiguousarray(
        W_proj.reshape(KDE, 128, DF).transpose(1, 0, 2)).astype(BF16)
    wx_arr = np.ascontiguousarray(
        W_x.T.reshape(KDF, 128, G4).transpose(1, 0, 2)).astype(BF16)
    whh_arr = np.ascontiguousarray(
        W_hh_p.T.reshape(KH, 128, G4).transpose(1, 0, 2)).astype(BF16)
    in_maps = []
    for c in range(NCORES):
        bsl = slice(c * BL, (c + 1) * BL)
        enc_c = outputs_encoder[bsl]                    # [64, 80, 1024]
        encT = np.ascontiguousarray(
            enc_c.transpose(2, 1, 0)                    # [1024, 80, 64]
            .reshape(KDE, 128, T * BL)
            .transpose(1, 0, 2)).astype(BF16)           # [128, 8, 5120]
        # gate-major bias [2048, 64] -> [128, m(16), 64], repeated CH x in t
        bgm = biasw[bsl].T.reshape(MT, 128, BL).transpose(1, 0, 2)  # [128,16,64]
        biasr = np.broadcast_to(bgm[:, :, None, :], (128, MT, CH, BL))
        biasr = np.ascontiguousarray(biasr).reshape(128, MT * NCOL).astype(BF16)
        in_maps.append({
            "ident": np.eye(128, dtype=np.float32).astype(BF16),
            "encT": encT,
            "wproj": wproj_arr,
            "wx": wx_arr,
            "whh": whh_arr,
            "biasr": biasr,
        })
    return in_maps


def run_on_device(in_maps, trace=False):
    from concourse.bass_utils import run_bass_kernel_spmd
    nc = _get_compiled()
    return run_bass_kernel_spmd(
        nc, in_maps, core_ids=list(range(NCORES)), trace=trace)


def kernel(**inputs):
    in_maps = _prep_maps(**inputs)
    res = run_on_device(in_maps)
    out = np.empty((B, T, H), np.float32)
    for c in range(NCORES):
        hT = res.results[c]["hT"]                       # [128, 4, 80, 64]
        out[c * BL:(c + 1) * BL] = (
            hT.transpose(3, 2, 1, 0).reshape(BL, T, H))
    return out
